# revision 5
# baseline (speedup 1.0000x reference)
"""Trainium2 Bass kernel for BERTSpanNER boundary scores.

Computes, for each batch b and label l:
    tag_logits = x @ W + b ; lm = log_softmax(tag_logits)
    inside/begin/end per label from the I,B,L,U tag groups
    cum = exclusive cumsum of inside over seq
    out[b,i,j,l] = min(cum[j+1]-cum[i], -EPS, begin[i], end[j])  (triu, else -1e9)

Sharding: 8 cores = 4 batches x 2 label-halves (8 labels each). Every core runs
the identical SPMD graph; per-core data differs only via the inputs (the batch
slice of x and a label-permuted copy of the W columns).
"""
import os
import sys

for _p in ("/opt/trn_rl_repo", "/root/.axon_site/_ro/trn_rl_repo"):
    if os.path.isdir(_p) and _p not in sys.path:
        sys.path.insert(0, _p)

import numpy as np
import concourse.bacc as bacc
import concourse.mybir as mybir
from concourse.tile import TileContext
from concourse.bass_utils import run_bass_kernel_spmd
from concourse.alu_op_type import AluOpType

F32 = mybir.dt.float32
AF = mybir.ActivationFunctionType

B, S, H, NL = 4, 1024, 400, 16
NT = 1 + 4 * NL          # 65
EPS = 1e-8
NEG = -1e9
P = 128
NST = S // P             # 8 seq tiles
LC = NL // 2             # 8 labels per core
KT = [101, 100, 100, 100]  # k-tiling of H+1=401
JC = 512                 # j chunk width (cols) for the span sweep
SECT = 512               # broadcast matmul moving width

_CACHED_NC = None


def _build():
    nc = bacc.Bacc()
    xTb = nc.declare_dram_parameter("xTb", [H + 1, S], F32, isOutput=False)
    Wz = nc.declare_dram_parameter("Wz", [H + 1, NT], F32, isOutput=False)
    Wlab = nc.declare_dram_parameter("Wlab", [H + 1, 4 * LC], F32, isOutput=False)
    eye = nc.declare_dram_parameter("eye", [P, P], F32, isOutput=False)
    ut = nc.declare_dram_parameter("ut", [P, P], F32, isOutput=False)    # ut[k,i]=1 if k<i
    idl = nc.declare_dram_parameter("idl", [LC, LC * P], F32, isOutput=False)
    mask8 = nc.declare_dram_parameter("mask8", [P, LC * P], F32, isOutput=False)
    out = nc.declare_dram_parameter("out", [S, S * LC], F32, isOutput=True)

    with TileContext(nc) as tc:
        with tc.tile_pool(name="const", bufs=1) as cpool, \
             tc.tile_pool(name="work", bufs=1) as wpool, \
             tc.tile_pool(name="xt", bufs=2) as xpool, \
             tc.tile_pool(name="sm", bufs=2) as smpool, \
             tc.tile_pool(name="u", bufs=2) as upool, \
             tc.tile_pool(name="oc", bufs=2) as opool, \
             tc.tile_pool(name="ps_small", bufs=4, space="PSUM") as pss, \
             tc.tile_pool(name="ps_b", bufs=2, space="PSUM") as psb:

            # ---------------- constants / inputs ----------------
            eye_sb = cpool.tile([P, P], F32, tag="eye")
            nc.sync.dma_start(out=eye_sb[:], in_=eye[:])
            ut_sb = cpool.tile([P, P], F32, tag="ut")
            nc.sync.dma_start(out=ut_sb[:], in_=ut[:])
            idl_sb = cpool.tile([LC, LC * P], F32, tag="idl")
            nc.sync.dma_start(out=idl_sb[:], in_=idl[:])
            mask_sb = cpool.tile([P, LC * P], F32, tag="mask8")
            nc.sync.dma_start(out=mask_sb[:], in_=mask8[:])
            wz_tiles, wl_tiles = [], []
            _k0 = 0
            for _ki, _kt in enumerate(KT):
                wz_k = cpool.tile([_kt, NT], F32, tag="wz%d" % _ki)
                nc.sync.dma_start(out=wz_k[:], in_=Wz[_k0:_k0 + _kt, :])
                wz_tiles.append(wz_k)
                wl_k = cpool.tile([_kt, 4 * LC], F32, tag="wl%d" % _ki)
                nc.sync.dma_start(out=wl_k[:], in_=Wlab[_k0:_k0 + _kt, :])
                wl_tiles.append(wl_k)
                _k0 += _kt

            neg_sb = cpool.tile([P, 1024], F32, tag="negt")
            nc.vector.memset(neg_sb[:], NEG)
            ones_row = cpool.tile([1, P], F32, tag="ones_row")   # lhsT for K=1 bcast
            nc.vector.memset(ones_row[:], 1.0)
            ones_col = cpool.tile([P, 1], F32, tag="ones_col")   # lhsT for colsum
            nc.vector.memset(ones_col[:], 1.0)

            # ---------------- NEG lower-triangle rectangles (independent) --------
            out3 = out[:].rearrange("(t p) f -> t p f", p=P)
            for t in range(1, NST):
                i0 = t * P
                reps = (i0 * LC) // 1024
                dst = out3[t, :, 0:i0 * LC].rearrange("p (r f) -> p r f", f=1024)
                src = neg_sb[:].rearrange("p (r f) -> p r f", r=1).broadcast_to((P, reps, 1024))
                nc.sync.dma_start(out=dst, in_=src)

            # ---------------- prologue: projection + log-softmax pieces ---------
            # per seq tile: inside/begin(G)/end(E2) (P, LC); cumsum C; transposed rows
            C_all = wpool.tile([P, NST * LC], F32, tag="c_all")
            G_all = wpool.tile([P, NST * LC], F32, tag="g_all")
            ins_all = wpool.tile([P, NST * LC], F32, tag="ins_all")
            E2_all = wpool.tile([P, NST * LC], F32, tag="e2_all")
            A_colT = wpool.tile([LC, S], F32, tag="a_colt")
            E2_colT = wpool.tile([LC, S], F32, tag="e2_colt")

            # load xTb in k tiles (double buffered)
            k0 = 0
            xk_tiles = []
            for kt in KT:
                xk = xpool.tile([kt, S], F32, tag="xk%d" % k0)
                nc.sync.dma_start(out=xk[:], in_=xTb[k0:k0 + kt, :])
                xk_tiles.append((k0, kt, xk))
                k0 += kt

            for t in range(NST):
                sl = slice(t * P, (t + 1) * P)
                ps65 = pss.tile([P, 512], F32, tag="ps_small")
                pslab = pss.tile([P, 512], F32, tag="ps_small")
                for ki, (k0, kt, xk) in enumerate(xk_tiles):
                    st, sp = ki == 0, ki == len(KT) - 1
                    nc.tensor.matmul(ps65[:, :NT], xk[:, sl], wz_tiles[ki][:],
                                     start=st, stop=sp)
                for ki, (k0, kt, xk) in enumerate(xk_tiles):
                    st, sp = ki == 0, ki == len(KT) - 1
                    nc.tensor.matmul(pslab[:, :4 * LC], xk[:, sl], wl_tiles[ki][:],
                                     start=st, stop=sp)

                rowmax = smpool.tile([P, 1], F32, tag="rowmax")
                nc.vector.tensor_reduce(rowmax[:], ps65[:, :NT], mybir.AxisListType.X,
                                        AluOpType.max)
                nrm = smpool.tile([P, 1], F32, tag="nrm")
                nc.vector.tensor_scalar(nrm[:], rowmax[:], -1.0, None, AluOpType.mult)

                e65 = smpool.tile([P, NT], F32, tag="e65")
                nc.scalar.activation(e65[:], ps65[:, :NT], AF.Exp, bias=nrm[:])
                elab = smpool.tile([P, 4 * LC], F32, tag="elab")
                nc.scalar.activation(elab[:], pslab[:, :4 * LC], AF.Exp, bias=nrm[:])

                ssum = smpool.tile([P, 1], F32, tag="ssum")
                nc.vector.tensor_reduce(ssum[:], e65[:], mybir.AxisListType.X,
                                        AluOpType.add)
                rs = smpool.tile([P, 1], F32, tag="rs")
                nc.vector.reciprocal(rs[:], ssum[:])

                el = elab[:].rearrange("p (l k) -> p l k", k=4)
                t01 = smpool.tile([P, LC], F32, tag="t01")
                nc.vector.tensor_tensor(t01[:], el[:, :, 0], el[:, :, 1], AluOpType.add)
                t23 = smpool.tile([P, LC], F32, tag="t23")
                nc.vector.tensor_tensor(t23[:], el[:, :, 2], el[:, :, 3], AluOpType.add)
                sum4 = smpool.tile([P, LC], F32, tag="sum4")
                nc.vector.tensor_tensor(sum4[:], t01[:], t23[:], AluOpType.add)
                beginE = smpool.tile([P, LC], F32, tag="beginE")
                nc.vector.tensor_tensor(beginE[:], el[:, :, 1], el[:, :, 3], AluOpType.add)
                endE = smpool.tile([P, LC], F32, tag="endE")
                nc.vector.tensor_tensor(endE[:], el[:, :, 2], el[:, :, 3], AluOpType.add)

                csl = slice(t * LC, (t + 1) * LC)
                nc.scalar.activation(ins_all[:, csl], sum4[:], AF.Ln, scale=rs[:])
                nc.scalar.activation(G_all[:, csl], beginE[:], AF.Ln, scale=rs[:])
                lend = smpool.tile([P, LC], F32, tag="lend")
                nc.scalar.activation(lend[:], endE[:], AF.Ln, scale=rs[:])
                nc.vector.tensor_scalar(E2_all[:, csl], lend[:], -EPS, None, AluOpType.min)

            # ---------------- cumsum over seq (exclusive) -----------------------
            carry = []
            for i in range(2):
                carry_t = wpool.tile([1, LC], F32, tag="carry%d" % i)
                carry.append(carry_t)
            nc.vector.memset(carry[0][:], 0.0)
            for t in range(NST):
                csl = slice(t * LC, (t + 1) * LC)
                cum_ps = pss.tile([P, 512], F32, tag="ps_small")
                nc.tensor.matmul(cum_ps[:, :LC], ut_sb[:], ins_all[:, csl],
                                 start=True, stop=False)
                nc.tensor.matmul(cum_ps[:, :LC], ones_row[:], carry[t % 2][:],
                                 start=False, stop=True)
                nc.scalar.activation(C_all[:, csl], cum_ps[:, :LC], AF.Copy)
                if t + 1 < NST:
                    cs_ps = pss.tile([P, 512], F32, tag="ps_small")
                    nc.tensor.matmul(cs_ps[:1, :LC], ones_col[:], ins_all[:, csl],
                                     start=True, stop=True)
                    nc.vector.tensor_tensor(carry[(t + 1) % 2][:], carry[t % 2][:],
                                            cs_ps[:1, :LC], AluOpType.add)

            # A = inclusive cumsum = C + inside ; transpose A and E2 to (LC, S)
            A_t = wpool.tile([P, NST * LC], F32, tag="a_incl")
            nc.vector.tensor_tensor(A_t[:], C_all[:], ins_all[:], AluOpType.add)
            for t in range(NST):
                csl = slice(t * LC, (t + 1) * LC)
                tp = pss.tile([P, 512], F32, tag="ps_small")
                nc.tensor.transpose(tp[:LC, :P], A_t[:, csl], eye_sb[:])
                nc.scalar.activation(A_colT[:, t * P:(t + 1) * P], tp[:LC, :P], AF.Copy)
                tp2 = pss.tile([P, 512], F32, tag="ps_small")
                nc.tensor.transpose(tp2[:LC, :P], E2_all[:, csl], eye_sb[:])
                nc.scalar.activation(E2_colT[:, t * P:(t + 1) * P], tp2[:LC, :P], AF.Copy)

            # ---------------- broadcast A,E2 rows across partitions -------------
            # A_b[p, l*S + j] = A[j, l]  (l-major), same for E2_b
            A_b = wpool.tile([P, LC * S], F32, tag="a_b")
            E2_b = wpool.tile([P, LC * S], F32, tag="e2_b")
            for srcT, dstB in ((A_colT, A_b), (E2_colT, E2_b)):
                for l in range(LC):
                    for jb in range(S // SECT):
                        bp = psb.tile([P, 1024], F32, tag="ps_b")
                        nc.tensor.matmul(bp[:, :SECT],
                                         idl_sb[:, l * P:(l + 1) * P],
                                         srcT[:, jb * SECT:(jb + 1) * SECT],
                                         start=True, stop=True)
                        nc.scalar.activation(
                            dstB[:, l * S + jb * SECT: l * S + (jb + 1) * SECT],
                            bp[:, :SECT], AF.Copy)

            A_b3 = A_b[:].rearrange("p (l j) -> p l j", l=LC)
            E2_b3 = E2_b[:].rearrange("p (l j) -> p l j", l=LC)

            # ---------------- main span sweep -----------------------------------
            for t in range(NST):
                i0 = t * P
                csl = slice(t * LC, (t + 1) * LC)
                # diag-block E2m = min(E2, trimask)  (l-major, width P)
                e2m = upool.tile([P, LC * P], F32, tag="e2m")
                nc.vector.tensor_tensor(e2m[:], mask_sb[:], E2_b3[:, :, i0:i0 + P],
                                        AluOpType.min)
                # chunks: [i0, i0+128) diag, then JC-wide to S
                bounds = [i0, i0 + P]
                while bounds[-1] < S:
                    bounds.append(min(bounds[-1] + JC, S))
                for ci in range(len(bounds) - 1):
                    ja, jb = bounds[ci], bounds[ci + 1]
                    cl = jb - ja
                    u = upool.tile([P, LC * cl], F32, tag="u")
                    for l in range(LC):
                        nc.vector.tensor_scalar(
                            u[:, l * cl:(l + 1) * cl],
                            A_b[:, l * S + ja: l * S + jb],
                            C_all[:, t * LC + l: t * LC + l + 1],
                            G_all[:, t * LC + l: t * LC + l + 1],
                            AluOpType.subtract, AluOpType.min)
                    oc = opool.tile([P, cl * LC], F32, tag="oc")
                    oc_ap = oc[:].rearrange("p (j l) -> p j l", l=LC)
                    u_ap = u[:].rearrange("p (l j) -> p j l", j=cl)
                    if ci == 0:
                        e2_ap = e2m[:].rearrange("p (l j) -> p j l", j=cl)
                    else:
                        e2_ap = E2_b3[:, :, ja:jb].rearrange("p l j -> p j l")
                    nc.vector.tensor_tensor(oc_ap, u_ap, e2_ap, AluOpType.min)
                    nc.sync.dma_start(out=out3[t, :, ja * LC:jb * LC], in_=oc[:])

    nc.compile()
    return nc


def _host_inputs(x, W, b):
    """Build per-core input maps. Core c: batch c//2, label half c%2."""
    x = np.asarray(x, dtype=np.float32)
    W = np.asarray(W, dtype=np.float32)
    b = np.asarray(b, dtype=np.float32)

    Wb = np.concatenate([W, b[None, :]], axis=0)          # (401, 65)
    eye = np.eye(P, dtype=np.float32)
    ut = np.triu(np.ones((P, P), np.float32), k=1)        # ut[k,i]=1 iff i>k
    idl = np.zeros((LC, LC * P), np.float32)
    for l in range(LC):
        idl[l, l * P:(l + 1) * P] = 1.0
    # mask8[i, l*P + j] = +big if j >= i else NEG  (strict lower -> NEG)
    jj = np.arange(P)[None, :] >= np.arange(P)[:, None]
    m = np.where(jj, np.float32(1e30), np.float32(NEG)).astype(np.float32)
    mask8 = np.tile(m, (1, LC))

    in_maps = []
    for c in range(8):
        bb, h = c // 2, c % 2
        cols = []
        for l in range(LC):
            base = 1 + 4 * (h * LC + l)
            cols.extend(range(base, base + 4))
        xTb = np.concatenate([x[bb].T, np.ones((1, S), np.float32)], axis=0)
        in_maps.append({
            "xTb": np.ascontiguousarray(xTb),
            "Wz": Wb,
            "Wlab": np.ascontiguousarray(Wb[:, cols]),
            "eye": eye, "ut": ut, "idl": idl, "mask8": mask8,
        })
    return in_maps


def kernel(x, mask, W, b, _collect=None):
    global _CACHED_NC
    if _CACHED_NC is None:
        _CACHED_NC = _build()
    nc = _CACHED_NC
    in_maps = _host_inputs(x, W, b)
    res = run_bass_kernel_spmd(nc, in_maps, list(range(8)))
    if _collect is not None:
        _collect.append(res)
    outf = np.empty((B, S, S, NL), dtype=np.float32)
    for c in range(8):
        bb, h = c // 2, c % 2
        outf[bb, :, :, h * LC:(h + 1) * LC] = res.results[c]["out"].reshape(S, S, LC)
    return outf


# revision 6
# speedup vs baseline: 1.4503x; 1.4503x over previous
"""Trainium2 Bass kernel for BERTSpanNER boundary scores.

out[b,i,j,l] = min(cum[j+1,l]-cum[i,l], -EPS, begin[i,l], end[j,l]) on the
upper triangle (j>=i), else -1e9, where cum/begin/end derive from
log_softmax(x @ W + b) per label's I,B,L,U tag group.

Sharding: 8 cores = 4 batches x 2 label-halves (8 labels each). All cores run
one identical SPMD graph; per-core work differs only through input data (the
batch slice of x, and a label-permuted copy of W's columns).

Device layout: per-core output is (S, LC, S) "l-major" [i, l, j] in bf16; the
host transposes to [i, j, l] and upcasts to f32 while scattering into the
final (B, S, S, NL) array.
"""
import os
import sys

for _p in ("/opt/trn_rl_repo", "/root/.axon_site/_ro/trn_rl_repo"):
    if os.path.isdir(_p) and _p not in sys.path:
        sys.path.insert(0, _p)

import numpy as np
import concourse.bacc as bacc
import concourse.mybir as mybir
from concourse.tile import TileContext
from concourse.bass_utils import run_bass_kernel_spmd
from concourse.alu_op_type import AluOpType

F32 = mybir.dt.float32
BF16 = mybir.dt.bfloat16
AF = mybir.ActivationFunctionType

B, S, H, NL = 4, 1024, 400, 16
NT = 1 + 4 * NL          # 65
EPS = 1e-8
NEG = -1e9
P = 128
NST = S // P             # 8 seq tiles
LC = NL // 2             # 8 labels per core
KT = [101, 100, 100, 100]  # k-tiling of H+1=401
SECT = 512               # broadcast matmul moving width (one PSUM bank)

OUT_DT = BF16            # device output dtype (host upcasts)
OUT_NP = np.dtype("uint16")  # raw view for bf16 transport

_CACHED_NC = None


def _build():
    nc = bacc.Bacc()
    xTb = nc.declare_dram_parameter("xTb", [H + 1, S], F32, isOutput=False)
    Wz = nc.declare_dram_parameter("Wz", [H + 1, NT], F32, isOutput=False)
    Wlab = nc.declare_dram_parameter("Wlab", [H + 1, 4 * LC], F32, isOutput=False)
    eye = nc.declare_dram_parameter("eye", [P, P], F32, isOutput=False)
    ut = nc.declare_dram_parameter("ut", [P, P], F32, isOutput=False)    # ut[k,i]=1 if k<i
    idl = nc.declare_dram_parameter("idl", [LC, LC * P], F32, isOutput=False)
    mask8 = nc.declare_dram_parameter("mask8", [P, LC * P], OUT_DT, isOutput=False)
    out = nc.declare_dram_parameter("out", [S, LC * S], OUT_DT, isOutput=True)

    with TileContext(nc) as tc:
        with tc.tile_pool(name="const", bufs=1) as cpool, \
             tc.tile_pool(name="work", bufs=1) as wpool, \
             tc.tile_pool(name="sm", bufs=8) as smpool, \
             tc.tile_pool(name="u", bufs=2) as upool, \
             tc.tile_pool(name="oc", bufs=2) as opool, \
             tc.tile_pool(name="ps_small", bufs=4, space="PSUM") as pss, \
             tc.tile_pool(name="ps_b", bufs=2, space="PSUM") as psb:

            # ---------------- input loads (HWDGE/sync ring, issued first) -----
            xk_tiles = []
            k0 = 0
            for ki, kt in enumerate(KT):
                xk = cpool.tile([kt, S], F32, tag="xk%d" % ki)
                nc.sync.dma_start(out=xk[:], in_=xTb[k0:k0 + kt, :])
                xk_tiles.append(xk)
                k0 += kt
            wz_tiles, wl_tiles = [], []
            k0 = 0
            for ki, kt in enumerate(KT):
                wz_k = cpool.tile([kt, NT], F32, tag="wz%d" % ki)
                nc.sync.dma_start(out=wz_k[:], in_=Wz[k0:k0 + kt, :])
                wz_tiles.append(wz_k)
                wl_k = cpool.tile([kt, 4 * LC], F32, tag="wl%d" % ki)
                nc.sync.dma_start(out=wl_k[:], in_=Wlab[k0:k0 + kt, :])
                wl_tiles.append(wl_k)
                k0 += kt
            eye_sb = cpool.tile([P, P], F32, tag="eye")
            nc.sync.dma_start(out=eye_sb[:], in_=eye[:])
            ut_sb = cpool.tile([P, P], F32, tag="ut")
            nc.sync.dma_start(out=ut_sb[:], in_=ut[:])
            idl_sb = cpool.tile([LC, LC * P], F32, tag="idl")
            nc.sync.dma_start(out=idl_sb[:], in_=idl[:])
            mask_sb = cpool.tile([P, LC * P], OUT_DT, tag="mask8")
            nc.sync.dma_start(out=mask_sb[:], in_=mask8[:])

            neg_sb = cpool.tile([P, S], OUT_DT, tag="negt")
            nc.vector.memset(neg_sb[:], NEG)
            ones_row = cpool.tile([1, P], F32, tag="ones_row")
            nc.vector.memset(ones_row[:], 1.0)
            ones_col = cpool.tile([P, 1], F32, tag="ones_col")
            nc.vector.memset(ones_col[:], 1.0)

            # ---------------- NEG lower-triangle (SWDGE ring, off to the side)
            out3 = out[:].rearrange("(t p) f -> t p f", p=P)
            for t in range(1, NST):
                i0 = t * P
                dst = out3[t, :, :].rearrange("p (l j) -> p l j", l=LC)[:, :, 0:i0]
                src = neg_sb[:, 0:i0].rearrange("p (r f) -> p r f", r=1) \
                    .broadcast_to((P, LC, i0))
                nc.gpsimd.dma_start(out=dst, in_=src)

            # ---------------- prologue phase 1: matmul + exp + partial sums ---
            C_all = wpool.tile([P, NST * LC], F32, tag="c_all")
            G_all = wpool.tile([P, NST * LC], F32, tag="g_all")
            ins_all = wpool.tile([P, NST * LC], F32, tag="ins_all")
            E2_all = wpool.tile([P, NST * LC], F32, tag="e2_all")
            A_colT = wpool.tile([LC, S], F32, tag="a_colt")
            E2_colT = wpool.tile([LC, S], F32, tag="e2_colt")
            sum4_all = wpool.tile([P, NST * LC], F32, tag="sum4_all")
            begE_all = wpool.tile([P, NST * LC], F32, tag="bege_all")
            endE_all = wpool.tile([P, NST * LC], F32, tag="ende_all")
            rs_all = wpool.tile([P, NST], F32, tag="rs_all")

            for t in range(NST):
                sl = slice(t * P, (t + 1) * P)
                csl = slice(t * LC, (t + 1) * LC)
                ps65 = pss.tile([P, 512], F32, tag="ps_small")
                pslab = pss.tile([P, 512], F32, tag="ps_small")
                for ki, xk in enumerate(xk_tiles):
                    st, sp = ki == 0, ki == len(KT) - 1
                    nc.tensor.matmul(ps65[:, :NT], xk[:, sl], wz_tiles[ki][:],
                                     start=st, stop=sp)
                for ki, xk in enumerate(xk_tiles):
                    st, sp = ki == 0, ki == len(KT) - 1
                    nc.tensor.matmul(pslab[:, :4 * LC], xk[:, sl], wl_tiles[ki][:],
                                     start=st, stop=sp)

                rowmax = smpool.tile([P, 1], F32, tag="rowmax")
                nc.vector.tensor_reduce(rowmax[:], ps65[:, :NT], mybir.AxisListType.X,
                                        AluOpType.max)
                nrm = smpool.tile([P, 1], F32, tag="nrm")
                nc.vector.tensor_scalar(nrm[:], rowmax[:], -1.0, None, AluOpType.mult)

                e65 = smpool.tile([P, NT], F32, tag="e65")
                nc.scalar.activation(e65[:], ps65[:, :NT], AF.Exp, bias=nrm[:])
                elab = smpool.tile([P, 4 * LC], F32, tag="elab")
                nc.scalar.activation(elab[:], pslab[:, :4 * LC], AF.Exp, bias=nrm[:])

                ssum = smpool.tile([P, 1], F32, tag="ssum")
                nc.vector.tensor_reduce(ssum[:], e65[:], mybir.AxisListType.X,
                                        AluOpType.add)
                nc.vector.reciprocal(rs_all[:, t:t + 1], ssum[:])

                el = elab[:].rearrange("p (l k) -> p l k", k=4)
                t01 = smpool.tile([P, LC], F32, tag="t01")
                nc.vector.tensor_tensor(t01[:], el[:, :, 0], el[:, :, 1], AluOpType.add)
                t23 = smpool.tile([P, LC], F32, tag="t23")
                nc.vector.tensor_tensor(t23[:], el[:, :, 2], el[:, :, 3], AluOpType.add)
                nc.vector.tensor_tensor(sum4_all[:, csl], t01[:], t23[:], AluOpType.add)
                nc.vector.tensor_tensor(begE_all[:, csl], el[:, :, 1], el[:, :, 3],
                                        AluOpType.add)
                nc.vector.tensor_tensor(endE_all[:, csl], el[:, :, 2], el[:, :, 3],
                                        AluOpType.add)

            # ---------------- prologue phase 2: all the Ln's together ---------
            for t in range(NST):
                csl = slice(t * LC, (t + 1) * LC)
                rs = rs_all[:, t:t + 1]
                nc.scalar.activation(ins_all[:, csl], sum4_all[:, csl], AF.Ln, scale=rs)
                nc.scalar.activation(G_all[:, csl], begE_all[:, csl], AF.Ln, scale=rs)
                lend = smpool.tile([P, LC], F32, tag="lend")
                nc.scalar.activation(lend[:], endE_all[:, csl], AF.Ln, scale=rs)
                nc.vector.tensor_scalar(E2_all[:, csl], lend[:], -EPS, None,
                                        AluOpType.min)

            # ---------------- E2 transpose + broadcast (independent of cumsum)
            E2_b = wpool.tile([P, LC * S], BF16, tag="e2_b")
            A_b = wpool.tile([P, LC * S], F32, tag="a_b")
            for t in range(NST):
                csl = slice(t * LC, (t + 1) * LC)
                tp2 = pss.tile([P, 512], F32, tag="ps_small")
                nc.tensor.transpose(tp2[:LC, :P], E2_all[:, csl], eye_sb[:])
                nc.scalar.activation(E2_colT[:, t * P:(t + 1) * P], tp2[:LC, :P],
                                     AF.Copy)
            for l in range(LC):
                for jb in range(S // SECT):
                    bp = psb.tile([P, SECT], F32, tag="ps_b")
                    nc.tensor.matmul(bp[:], idl_sb[:, l * P:(l + 1) * P],
                                     E2_colT[:, jb * SECT:(jb + 1) * SECT],
                                     start=True, stop=True)
                    nc.scalar.activation(
                        E2_b[:, l * S + jb * SECT: l * S + (jb + 1) * SECT],
                        bp[:], AF.Copy)

            # ---------------- cumsum over seq (exclusive) + A broadcast -------
            carry = []
            for i in range(2):
                carry_t = wpool.tile([1, LC], F32, tag="carry%d" % i)
                carry.append(carry_t)
            nc.vector.memset(carry[0][:], 0.0)
            for t in range(NST):
                csl = slice(t * LC, (t + 1) * LC)
                cum_ps = pss.tile([P, 512], F32, tag="ps_small")
                nc.tensor.matmul(cum_ps[:, :LC], ut_sb[:], ins_all[:, csl],
                                 start=True, stop=False)
                nc.tensor.matmul(cum_ps[:, :LC], ones_row[:], carry[t % 2][:],
                                 start=False, stop=True)
                nc.scalar.activation(C_all[:, csl], cum_ps[:, :LC], AF.Copy)
                if t + 1 < NST:
                    cs_ps = pss.tile([P, 512], F32, tag="ps_small")
                    nc.tensor.matmul(cs_ps[:1, :LC], ones_col[:], ins_all[:, csl],
                                     start=True, stop=True)
                    nc.vector.tensor_tensor(carry[(t + 1) % 2][:], carry[t % 2][:],
                                            cs_ps[:1, :LC], AluOpType.add)

            A_incl = wpool.tile([P, NST * LC], F32, tag="a_incl")
            nc.vector.tensor_tensor(A_incl[:], C_all[:], ins_all[:], AluOpType.add)
            for t in range(NST):
                csl = slice(t * LC, (t + 1) * LC)
                tp = pss.tile([P, 512], F32, tag="ps_small")
                nc.tensor.transpose(tp[:LC, :P], A_incl[:, csl], eye_sb[:])
                nc.scalar.activation(A_colT[:, t * P:(t + 1) * P], tp[:LC, :P], AF.Copy)
            for l in range(LC):
                for jb in range(S // SECT):
                    bp = psb.tile([P, SECT], F32, tag="ps_b")
                    nc.tensor.matmul(bp[:], idl_sb[:, l * P:(l + 1) * P],
                                     A_colT[:, jb * SECT:(jb + 1) * SECT],
                                     start=True, stop=True)
                    nc.scalar.activation(A_b[:, l * S + jb * SECT: l * S + (jb + 1) * SECT],
                                         bp[:], AF.Copy)

            # ---------------- main span sweep (l-major, bf16) ----------------
            E2_b3 = E2_b[:].rearrange("p (l j) -> p l j", l=LC)
            for t in range(NST):
                i0 = t * P
                W = S - i0
                # diag-block E2m = min(E2, trimask)
                e2m = upool.tile([P, LC * P], OUT_DT, tag="e2m")
                nc.vector.tensor_tensor(e2m[:], mask_sb[:], E2_b3[:, :, i0:i0 + P],
                                        AluOpType.min)
                # pass1: U[l-major] = min(A - C, G)  (per label: full [i0, S) range)
                u = upool.tile([P, LC * W], OUT_DT, tag="u")
                for l in range(LC):
                    nc.vector.tensor_scalar(
                        u[:, l * W:(l + 1) * W],
                        A_b[:, l * S + i0:(l + 1) * S],
                        C_all[:, t * LC + l: t * LC + l + 1],
                        G_all[:, t * LC + l: t * LC + l + 1],
                        AluOpType.subtract, AluOpType.min)
                # pass2: out = min(U, E2) ; diag chunk uses masked E2m
                oc = opool.tile([P, LC * W], OUT_DT, tag="oc")
                oc3 = oc[:].rearrange("p (l j) -> p l j", j=W)
                u3 = u[:].rearrange("p (l j) -> p l j", j=W)
                e2m3 = e2m[:].rearrange("p (l j) -> p l j", j=P)
                nc.vector.tensor_tensor(oc3[:, :, 0:P], u3[:, :, 0:P], e2m3,
                                        AluOpType.min)
                if W > P:
                    nc.vector.tensor_tensor(oc3[:, :, P:W], u3[:, :, P:W],
                                            E2_b3[:, :, i0 + P:S], AluOpType.min)
                dst = out3[t, :, :].rearrange("p (l j) -> p l j", l=LC)[:, :, i0:S]
                nc.sync.dma_start(out=dst, in_=oc3)

    nc.compile()
    return nc


def _host_inputs(x, W, b):
    """Build per-core input maps. Core c: batch c//2, label half c%2."""
    x = np.asarray(x, dtype=np.float32)
    W = np.asarray(W, dtype=np.float32)
    b = np.asarray(b, dtype=np.float32)

    Wb = np.concatenate([W, b[None, :]], axis=0)          # (401, 65)
    eye = np.eye(P, dtype=np.float32)
    ut = np.triu(np.ones((P, P), np.float32), k=1)        # ut[k,i]=1 iff i>k
    idl = np.zeros((LC, LC * P), np.float32)
    for l in range(LC):
        idl[l, l * P:(l + 1) * P] = 1.0
    # mask8[i, l*P + j] = +big if j >= i else NEG  (l-major), in OUT_DT
    jj = np.arange(P)[None, :] >= np.arange(P)[:, None]
    m = np.where(jj, np.float32(1e30), np.float32(NEG)).astype(np.float32)
    m = _to_out_dt(np.tile(m, (1, LC)))

    in_maps = []
    for c in range(8):
        bb, h = c // 2, c % 2
        cols = []
        for l in range(LC):
            base = 1 + 4 * (h * LC + l)
            cols.extend(range(base, base + 4))
        xTb = np.concatenate([x[bb].T, np.ones((1, S), np.float32)], axis=0)
        in_maps.append({
            "xTb": np.ascontiguousarray(xTb),
            "Wz": Wb,
            "Wlab": np.ascontiguousarray(Wb[:, cols]),
            "eye": eye, "ut": ut, "idl": idl, "mask8": m,
        })
    return in_maps


def _to_out_dt(a):
    if OUT_DT == F32:
        return a.astype(np.float32)
    # f32 -> bf16 (round to nearest even), transported as uint16
    u = a.astype(np.float32).view(np.uint32)
    r = ((u >> 16) & 1) + 0x7FFF
    return ((u + r) >> 16).astype(np.uint16)


def _from_out_dt(a):
    if OUT_DT == F32:
        return a
    return (a.astype(np.uint32) << 16).view(np.float32)


def kernel(x, mask, W, b, _collect=None):
    global _CACHED_NC
    if _CACHED_NC is None:
        _CACHED_NC = _build()
    nc = _CACHED_NC
    in_maps = _host_inputs(x, W, b)
    res = run_bass_kernel_spmd(nc, in_maps, list(range(8)))
    if _collect is not None:
        _collect.append(res)
    outf = np.empty((B, S, S, NL), dtype=np.float32)
    for c in range(8):
        bb, h = c // 2, c % 2
        o = res.results[c]["out"]
        if o.dtype != np.float32:
            o = _from_out_dt(o.view(OUT_NP) if o.dtype != OUT_NP else o)
        o = o.reshape(S, LC, S)                       # [i, l, j]
        outf[bb, :, :, h * LC:(h + 1) * LC] = o.transpose(0, 2, 1)
    return outf


# revision 7
# speedup vs baseline: 1.8973x; 1.3082x over previous
"""Trainium2 Bass kernel for BERTSpanNER boundary scores.

out[b,i,j,l] = min(cum[j+1,l]-cum[i,l], -EPS, begin[i,l], end[j,l]) on the
upper triangle (j>=i), else -1e9, where cum/begin/end derive from
log_softmax(x @ W + b) per label's I,B,L,U tag group.

Sharding: 8 cores = 4 batches x 2 label-halves (8 labels each). All cores run
one identical SPMD graph; per-core work differs only through input data (the
batch slice of x, and a label-permuted copy of W's columns).

Device writes only the computed upper-triangle region in an l-major (S, LC, S)
bf16 layout; the constant -1e9 lower triangle is filled on the host, which
also transposes to [i, j, l] and upcasts to f32.
"""
import os
import sys

for _p in ("/opt/trn_rl_repo", "/root/.axon_site/_ro/trn_rl_repo"):
    if os.path.isdir(_p) and _p not in sys.path:
        sys.path.insert(0, _p)

import numpy as np
import concourse.bacc as bacc
import concourse.mybir as mybir
from concourse.bass import _add_dep_helper
from concourse.tile import TileContext
from concourse.bass_utils import run_bass_kernel_spmd
from concourse.alu_op_type import AluOpType

F32 = mybir.dt.float32
BF16 = mybir.dt.bfloat16
AF = mybir.ActivationFunctionType

B, S, H, NL = 4, 1024, 400, 16
NT = 1 + 4 * NL          # 65
EPS = 1e-8
NEG = -1e9
P = 128
NST = S // P             # 8 seq tiles
LC = NL // 2             # 8 labels per core
KT = [101, 100, 100, 100]  # k-tiling of H+1=401

OUT_DT = BF16            # device output dtype (host upcasts)
OUT_NP = np.dtype("uint16")

_CACHED_NC = None


def _build():
    nc = bacc.Bacc()
    xTb = nc.declare_dram_parameter("xTb", [H + 1, S], F32, isOutput=False)
    Wz = nc.declare_dram_parameter("Wz", [H + 1, NT], F32, isOutput=False)
    Wlab = nc.declare_dram_parameter("Wlab", [H + 1, 4 * LC], F32, isOutput=False)
    eye = nc.declare_dram_parameter("eye", [P, P], F32, isOutput=False)
    ut = nc.declare_dram_parameter("ut", [P, P], F32, isOutput=False)    # ut[k,i]=1 if k<i
    mask8 = nc.declare_dram_parameter("mask8", [P, LC * P], OUT_DT, isOutput=False)
    out = nc.declare_dram_parameter("out", [S, LC * S], OUT_DT, isOutput=True)

    a_row_d = nc.dram_tensor("a_row_d", [LC, S], F32)
    e2_row_d = nc.dram_tensor("e2_row_d", [LC, S], BF16)

    with TileContext(nc) as tc:
        with tc.tile_pool(name="const", bufs=1) as cpool, \
             tc.tile_pool(name="work", bufs=1) as wpool, \
             tc.tile_pool(name="sm", bufs=8) as smpool, \
             tc.tile_pool(name="u", bufs=2) as upool, \
             tc.tile_pool(name="oc", bufs=2) as opool, \
             tc.tile_pool(name="ps_small", bufs=4, space="PSUM") as pss:

            # ---------------- input loads ----------------
            xk_tiles = []
            k0 = 0
            for ki, kt in enumerate(KT):
                xk = cpool.tile([kt, S], F32, tag="xk%d" % ki)
                nc.sync.dma_start(out=xk[:], in_=xTb[k0:k0 + kt, :])
                xk_tiles.append(xk)
                k0 += kt
            wz_tiles, wl_tiles = [], []
            k0 = 0
            for ki, kt in enumerate(KT):
                wz_k = cpool.tile([kt, NT], F32, tag="wz%d" % ki)
                nc.sync.dma_start(out=wz_k[:], in_=Wz[k0:k0 + kt, :])
                wz_tiles.append(wz_k)
                wl_k = cpool.tile([kt, 4 * LC], F32, tag="wl%d" % ki)
                nc.sync.dma_start(out=wl_k[:], in_=Wlab[k0:k0 + kt, :])
                wl_tiles.append(wl_k)
                k0 += kt
            eye_sb = cpool.tile([P, P], F32, tag="eye")
            nc.sync.dma_start(out=eye_sb[:], in_=eye[:])
            ut_sb = cpool.tile([P, P], F32, tag="ut")
            nc.sync.dma_start(out=ut_sb[:], in_=ut[:])
            mask_sb = cpool.tile([P, LC * P], OUT_DT, tag="mask8")
            nc.sync.dma_start(out=mask_sb[:], in_=mask8[:])

            ones_row = cpool.tile([1, P], F32, tag="ones_row")
            nc.vector.memset(ones_row[:], 1.0)
            ones_col = cpool.tile([P, 1], F32, tag="ones_col")
            nc.vector.memset(ones_col[:], 1.0)

            # ---------------- prologue phase 1: matmul + exp + partial sums ---
            C_all = wpool.tile([P, NST * LC], F32, tag="c_all")
            G_all = wpool.tile([P, NST * LC], F32, tag="g_all")
            ins_all = wpool.tile([P, NST * LC], F32, tag="ins_all")
            E2_all = wpool.tile([P, NST * LC], F32, tag="e2_all")
            A_colT = wpool.tile([LC, S], F32, tag="a_colt")
            E2_colT = wpool.tile([LC, S], BF16, tag="e2_colt")
            sum4_all = wpool.tile([P, NST * LC], F32, tag="sum4_all")
            begE_all = wpool.tile([P, NST * LC], F32, tag="bege_all")
            endE_all = wpool.tile([P, NST * LC], F32, tag="ende_all")
            rs_all = wpool.tile([P, NST], F32, tag="rs_all")

            for t in range(NST):
                sl = slice(t * P, (t + 1) * P)
                csl = slice(t * LC, (t + 1) * LC)
                ps65 = pss.tile([P, 512], F32, tag="ps_small")
                pslab = pss.tile([P, 512], F32, tag="ps_small")
                for ki, xk in enumerate(xk_tiles):
                    st, sp = ki == 0, ki == len(KT) - 1
                    nc.tensor.matmul(ps65[:, :NT], xk[:, sl], wz_tiles[ki][:],
                                     start=st, stop=sp)
                for ki, xk in enumerate(xk_tiles):
                    st, sp = ki == 0, ki == len(KT) - 1
                    nc.tensor.matmul(pslab[:, :4 * LC], xk[:, sl], wl_tiles[ki][:],
                                     start=st, stop=sp)

                rowmax = smpool.tile([P, 1], F32, tag="rowmax")
                nc.vector.tensor_reduce(rowmax[:], ps65[:, :NT], mybir.AxisListType.X,
                                        AluOpType.max)
                nrm = smpool.tile([P, 1], F32, tag="nrm")
                nc.vector.tensor_scalar(nrm[:], rowmax[:], -1.0, None, AluOpType.mult)

                e65 = smpool.tile([P, NT], F32, tag="e65")
                nc.scalar.activation(e65[:], ps65[:, :NT], AF.Exp, bias=nrm[:])
                elab = smpool.tile([P, 4 * LC], F32, tag="elab")
                nc.scalar.activation(elab[:], pslab[:, :4 * LC], AF.Exp, bias=nrm[:])

                ssum = smpool.tile([P, 1], F32, tag="ssum")
                nc.vector.tensor_reduce(ssum[:], e65[:], mybir.AxisListType.X,
                                        AluOpType.add)
                nc.vector.reciprocal(rs_all[:, t:t + 1], ssum[:])

                el = elab[:].rearrange("p (l k) -> p l k", k=4)
                t01 = smpool.tile([P, LC], F32, tag="t01")
                nc.vector.tensor_tensor(t01[:], el[:, :, 0], el[:, :, 1], AluOpType.add)
                t23 = smpool.tile([P, LC], F32, tag="t23")
                nc.vector.tensor_tensor(t23[:], el[:, :, 2], el[:, :, 3], AluOpType.add)
                nc.vector.tensor_tensor(sum4_all[:, csl], t01[:], t23[:], AluOpType.add)
                nc.vector.tensor_tensor(begE_all[:, csl], el[:, :, 1], el[:, :, 3],
                                        AluOpType.add)
                nc.vector.tensor_tensor(endE_all[:, csl], el[:, :, 2], el[:, :, 3],
                                        AluOpType.add)

            # ---------------- prologue phase 2: all the Ln's ------------------
            for t in range(NST):
                csl = slice(t * LC, (t + 1) * LC)
                rs = rs_all[:, t:t + 1]
                nc.scalar.activation(ins_all[:, csl], sum4_all[:, csl], AF.Ln, scale=rs)
                nc.scalar.activation(G_all[:, csl], begE_all[:, csl], AF.Ln, scale=rs)
                lend = smpool.tile([P, LC], F32, tag="lend")
                nc.scalar.activation(lend[:], endE_all[:, csl], AF.Ln, scale=rs)
                nc.vector.tensor_scalar(E2_all[:, csl], lend[:], -EPS, None,
                                        AluOpType.min)

            # ---------------- E2 transpose + DRAM-broadcast -------------------
            E2_b = wpool.tile([P, LC * S], BF16, tag="e2_b")
            A_b = wpool.tile([P, LC * S], F32, tag="a_b")
            for t in range(NST):
                csl = slice(t * LC, (t + 1) * LC)
                tp2 = pss.tile([P, 512], F32, tag="ps_small")
                nc.tensor.transpose(tp2[:LC, :P], E2_all[:, csl], eye_sb[:])
                nc.scalar.activation(E2_colT[:, t * P:(t + 1) * P], tp2[:LC, :P],
                                     AF.Copy)
            dma_w_e2 = nc.sync.dma_start(out=e2_row_d[:], in_=E2_colT[:])
            dma_r_e2 = nc.sync.dma_start(
                out=E2_b[:], in_=e2_row_d[:].rearrange("l j -> (l j)").partition_broadcast(P))
            _add_dep_helper(dma_r_e2.ins, dma_w_e2.ins, True, "e2 row RAW via dram")

            # ---------------- cumsum over seq (exclusive), de-serialized ------
            # colsums for all tiles in one matmul -> (1, NST*LC)
            cs_ps = pss.tile([P, 512], F32, tag="ps_small")
            nc.tensor.matmul(cs_ps[:1, :NST * LC], ones_col[:], ins_all[:],
                             start=True, stop=True)
            cs_row = smpool.tile([1, NST * LC], F32, tag="cs_row")
            nc.scalar.activation(cs_row[:], cs_ps[:1, :NST * LC], AF.Copy)
            # inclusive prefix over t (log-shift adds), then use shifted reads
            pre = [cs_row]
            for lev, sh in enumerate((LC, 2 * LC, 4 * LC)):
                nxt = smpool.tile([1, NST * LC], F32, tag="pre%d" % lev)
                nc.vector.tensor_copy(nxt[:, :sh], pre[-1][:, :sh])
                nc.vector.tensor_tensor(nxt[:, sh:], pre[-1][:, sh:],
                                        pre[-1][:, :NST * LC - sh], AluOpType.add)
                pre.append(nxt)
            inc_pref = pre[-1]   # inclusive prefix of colsums over t

            for t in range(NST):
                csl = slice(t * LC, (t + 1) * LC)
                cum_ps = pss.tile([P, 512], F32, tag="ps_small")
                nc.tensor.matmul(cum_ps[:, :LC], ut_sb[:], ins_all[:, csl],
                                 start=True, stop=t != 0)
                if t > 0:
                    nc.tensor.matmul(cum_ps[:, :LC], ones_row[:],
                                     inc_pref[:, (t - 1) * LC: t * LC],
                                     start=False, stop=True)
                nc.scalar.activation(C_all[:, csl], cum_ps[:, :LC], AF.Copy)

            A_incl = wpool.tile([P, NST * LC], F32, tag="a_incl")
            nc.vector.tensor_tensor(A_incl[:], C_all[:], ins_all[:], AluOpType.add)
            for t in range(NST):
                csl = slice(t * LC, (t + 1) * LC)
                tp = pss.tile([P, 512], F32, tag="ps_small")
                nc.tensor.transpose(tp[:LC, :P], A_incl[:, csl], eye_sb[:])
                nc.scalar.activation(A_colT[:, t * P:(t + 1) * P], tp[:LC, :P], AF.Copy)
            dma_w_a = nc.sync.dma_start(out=a_row_d[:], in_=A_colT[:])
            dma_r_a = nc.sync.dma_start(
                out=A_b[:], in_=a_row_d[:].rearrange("l j -> (l j)").partition_broadcast(P))
            _add_dep_helper(dma_r_a.ins, dma_w_a.ins, True, "a row RAW via dram")

            # ---------------- main span sweep (l-major, bf16) ----------------
            out3 = out[:].rearrange("(t p) f -> t p f", p=P)
            E2_b3 = E2_b[:].rearrange("p (l j) -> p l j", l=LC)
            for t in range(NST):
                i0 = t * P
                W = S - i0
                e2m = upool.tile([P, LC * P], OUT_DT, tag="e2m")
                nc.vector.tensor_tensor(e2m[:], mask_sb[:], E2_b3[:, :, i0:i0 + P],
                                        AluOpType.min)
                u = upool.tile([P, LC * W], OUT_DT, tag="u")
                for l in range(LC):
                    nc.vector.tensor_scalar(
                        u[:, l * W:(l + 1) * W],
                        A_b[:, l * S + i0:(l + 1) * S],
                        C_all[:, t * LC + l: t * LC + l + 1],
                        G_all[:, t * LC + l: t * LC + l + 1],
                        AluOpType.subtract, AluOpType.min)
                oc = opool.tile([P, LC * W], OUT_DT, tag="oc")
                oc3 = oc[:].rearrange("p (l j) -> p l j", j=W)
                u3 = u[:].rearrange("p (l j) -> p l j", j=W)
                e2m3 = e2m[:].rearrange("p (l j) -> p l j", j=P)
                nc.vector.tensor_tensor(oc3[:, :, 0:P], u3[:, :, 0:P], e2m3,
                                        AluOpType.min)
                if W > P:
                    nc.vector.tensor_tensor(oc3[:, :, P:W], u3[:, :, P:W],
                                            E2_b3[:, :, i0 + P:S], AluOpType.min)
                dst = out3[t, :, :].rearrange("p (l j) -> p l j", l=LC)[:, :, i0:S]
                nc.sync.dma_start(out=dst, in_=oc3)

    nc.compile()
    return nc


def _host_inputs(x, W, b):
    """Build per-core input maps. Core c: batch c//2, label half c%2."""
    x = np.asarray(x, dtype=np.float32)
    W = np.asarray(W, dtype=np.float32)
    b = np.asarray(b, dtype=np.float32)

    Wb = np.concatenate([W, b[None, :]], axis=0)          # (401, 65)
    eye = np.eye(P, dtype=np.float32)
    ut = np.triu(np.ones((P, P), np.float32), k=1)        # ut[k,i]=1 iff i>k
    jj = np.arange(P)[None, :] >= np.arange(P)[:, None]
    m = np.where(jj, np.float32(1e30), np.float32(NEG)).astype(np.float32)
    m = _to_out_dt(np.tile(m, (1, LC)))

    in_maps = []
    for c in range(8):
        bb, h = c // 2, c % 2
        cols = []
        for l in range(LC):
            base = 1 + 4 * (h * LC + l)
            cols.extend(range(base, base + 4))
        xTb = np.concatenate([x[bb].T, np.ones((1, S), np.float32)], axis=0)
        in_maps.append({
            "xTb": np.ascontiguousarray(xTb),
            "Wz": Wb,
            "Wlab": np.ascontiguousarray(Wb[:, cols]),
            "eye": eye, "ut": ut, "mask8": m,
        })
    return in_maps


def _to_out_dt(a):
    if OUT_DT == F32:
        return a.astype(np.float32)
    u = a.astype(np.float32).view(np.uint32)
    r = ((u >> 16) & 1) + 0x7FFF
    return ((u + r) >> 16).astype(np.uint16)


def _from_out_dt(a):
    if OUT_DT == F32:
        return a
    return (a.astype(np.uint32) << 16).view(np.float32)


def kernel(x, mask, W, b, _collect=None):
    global _CACHED_NC
    if _CACHED_NC is None:
        _CACHED_NC = _build()
    nc = _CACHED_NC
    in_maps = _host_inputs(x, W, b)
    res = run_bass_kernel_spmd(nc, in_maps, list(range(8)))
    if _collect is not None:
        _collect.append(res)
    outf = np.empty((B, S, S, NL), dtype=np.float32)
    for c in range(8):
        bb, h = c // 2, c % 2
        o = res.results[c]["out"]
        if o.dtype != np.float32:
            o = _from_out_dt(o.view(OUT_NP) if o.dtype != OUT_NP else o)
        o = o.reshape(S, LC, S)                       # [i, l, j]
        outf[bb, :, :, h * LC:(h + 1) * LC] = o.transpose(0, 2, 1)
    # constant lower triangle filled on host (device writes only j >= i0 of
    # each row tile; below-diagonal within the tile is masked on device)
    for i in range(1, S):
        i0 = (i // P) * P
        if i0 > 0:
            outf[:, i, :i0, :] = NEG
    return outf


# revision 8
# speedup vs baseline: 2.2594x; 1.1908x over previous
"""Trainium2 Bass kernel for BERTSpanNER boundary scores.

out[b,i,j,l] = min(cum[j+1,l]-cum[i,l], -EPS, begin[i,l], end[j,l]) on the
upper triangle (j>=i), else -1e9, where cum/begin/end derive from
log_softmax(x @ W + b) per label's I,B,L,U tag group.

Sharding: 8 cores = 4 batches x 2 label-halves (8 labels each). All cores run
one identical SPMD graph; per-core work differs only through input data (the
batch slice of x, and a label-permuted copy of W's columns).

Device writes only the computed upper-triangle region in an l-major (S, LC, S)
bf16 layout; the constant -1e9 lower triangle is filled on the host, which
also transposes to [i, j, l] and upcasts to f32.
"""
import os
import sys

for _p in ("/opt/trn_rl_repo", "/root/.axon_site/_ro/trn_rl_repo"):
    if os.path.isdir(_p) and _p not in sys.path:
        sys.path.insert(0, _p)

import numpy as np
import concourse.bacc as bacc
import concourse.mybir as mybir
from concourse.bass import _add_dep_helper
from concourse.tile import TileContext
from concourse.bass_utils import run_bass_kernel_spmd
from concourse.alu_op_type import AluOpType

F32 = mybir.dt.float32
BF16 = mybir.dt.bfloat16
AF = mybir.ActivationFunctionType

B, S, H, NL = 4, 1024, 400, 16
NT = 1 + 4 * NL          # 65
EPS = 1e-8
NEG = -1e9
P = 128
NST = S // P             # 8 seq tiles
LC = NL // 2             # 8 labels per core
KT = [101, 100, 100, 100]  # k-tiling of H+1=401
ACT_SPLIT = 5            # labels 0..4 take the ScalarE subtract path

OUT_DT = BF16            # device output dtype (host upcasts)
OUT_NP = np.dtype("uint16")

_CACHED_NC = None


def _build():
    nc = bacc.Bacc()
    xTb = nc.declare_dram_parameter("xTb", [H + 1, S], F32, isOutput=False)
    Wcat = nc.declare_dram_parameter("Wcat", [H + 1, NT + 4 * LC], F32, isOutput=False)
    eye = nc.declare_dram_parameter("eye", [P, P], F32, isOutput=False)
    ut = nc.declare_dram_parameter("ut", [P, P], F32, isOutput=False)    # ut[k,i]=1 if k<i
    mask8 = nc.declare_dram_parameter("mask8", [P, LC * P], OUT_DT, isOutput=False)
    out = nc.declare_dram_parameter("out", [S, LC * S], OUT_DT, isOutput=True)

    a_row_d = nc.dram_tensor("a_row_d", [LC, S], F32)
    e2_row_d = nc.dram_tensor("e2_row_d", [LC, S], BF16)

    with TileContext(nc) as tc:
        with tc.tile_pool(name="const", bufs=1) as cpool, \
             tc.tile_pool(name="work", bufs=1) as wpool, \
             tc.tile_pool(name="sm", bufs=8) as smpool, \
             tc.tile_pool(name="u", bufs=2) as upool, \
             tc.tile_pool(name="oc", bufs=2) as opool, \
             tc.tile_pool(name="ps_small", bufs=4, space="PSUM") as pss:

            # ---------------- input loads ----------------
            xk_tiles = []
            k0 = 0
            for ki, kt in enumerate(KT):
                xk = cpool.tile([kt, S], F32, tag="xk%d" % ki)
                nc.sync.dma_start(out=xk[:], in_=xTb[k0:k0 + kt, :])
                xk_tiles.append(xk)
                k0 += kt
            wc_tiles = []
            k0 = 0
            NW = NT + 4 * LC
            for ki, kt in enumerate(KT):
                wc_k = cpool.tile([kt, NW], F32, tag="wc%d" % ki)
                nc.scalar.dma_start(out=wc_k[:], in_=Wcat[k0:k0 + kt, :])
                wc_tiles.append(wc_k)
                k0 += kt
            eye_sb = cpool.tile([P, P], F32, tag="eye")
            nc.scalar.dma_start(out=eye_sb[:], in_=eye[:])
            ut_sb = cpool.tile([P, P], F32, tag="ut")
            nc.gpsimd.dma_start(out=ut_sb[:], in_=ut[:])
            mask_sb = cpool.tile([P, LC * P], OUT_DT, tag="mask8")
            nc.gpsimd.dma_start(out=mask_sb[:], in_=mask8[:])

            ones_row = cpool.tile([1, P], F32, tag="ones_row")
            nc.vector.memset(ones_row[:], 1.0)
            ones_col = cpool.tile([P, 1], F32, tag="ones_col")
            nc.vector.memset(ones_col[:], 1.0)

            # ---------------- prologue phase 1: matmul + exp + partial sums ---
            C_all = wpool.tile([P, NST * LC], F32, tag="c_all")
            G_all = wpool.tile([P, NST * LC], F32, tag="g_all")
            ins_all = wpool.tile([P, NST * LC], F32, tag="ins_all")
            E2_all = wpool.tile([P, NST * LC], F32, tag="e2_all")
            A_colT = wpool.tile([LC, S], F32, tag="a_colt")
            E2_colT = wpool.tile([LC, S], BF16, tag="e2_colt")
            sum4_all = wpool.tile([P, NST * LC], F32, tag="sum4_all")
            begE_all = wpool.tile([P, NST * LC], F32, tag="bege_all")
            endE_all = wpool.tile([P, NST * LC], F32, tag="ende_all")
            rs_all = wpool.tile([P, NST], F32, tag="rs_all")

            for t in range(NST):
                sl = slice(t * P, (t + 1) * P)
                csl = slice(t * LC, (t + 1) * LC)
                ps97 = pss.tile([P, 512], F32, tag="ps_small")
                for ki, xk in enumerate(xk_tiles):
                    st, sp = ki == 0, ki == len(KT) - 1
                    nc.tensor.matmul(ps97[:, :NW], xk[:, sl], wc_tiles[ki][:],
                                     start=st, stop=sp)

                rowmax = smpool.tile([P, 1], F32, tag="rowmax")
                nc.vector.tensor_reduce(rowmax[:], ps97[:, :NT], mybir.AxisListType.X,
                                        AluOpType.max)
                nrm = smpool.tile([P, 1], F32, tag="nrm")
                nc.vector.tensor_scalar(nrm[:], rowmax[:], -1.0, None, AluOpType.mult)

                e97 = smpool.tile([P, NW], F32, tag="e97")
                nc.scalar.activation(e97[:], ps97[:, :NW], AF.Exp, bias=nrm[:])
                e65 = e97[:, :NT]
                elab = e97[:, NT:NW]

                ssum = smpool.tile([P, 1], F32, tag="ssum")
                nc.vector.tensor_reduce(ssum[:], e65[:], mybir.AxisListType.X,
                                        AluOpType.add)
                nc.vector.reciprocal(rs_all[:, t:t + 1], ssum[:])

                el = elab.rearrange("p (l k) -> p l k", k=4)
                t01 = smpool.tile([P, LC], F32, tag="t01")
                nc.vector.tensor_tensor(t01[:], el[:, :, 0], el[:, :, 1], AluOpType.add)
                t23 = smpool.tile([P, LC], F32, tag="t23")
                nc.vector.tensor_tensor(t23[:], el[:, :, 2], el[:, :, 3], AluOpType.add)
                nc.vector.tensor_tensor(sum4_all[:, csl], t01[:], t23[:], AluOpType.add)
                nc.vector.tensor_tensor(begE_all[:, csl], el[:, :, 1], el[:, :, 3],
                                        AluOpType.add)
                nc.vector.tensor_tensor(endE_all[:, csl], el[:, :, 2], el[:, :, 3],
                                        AluOpType.add)

            # ---------------- prologue phase 2: all the Ln's ------------------
            for t in range(NST):
                csl = slice(t * LC, (t + 1) * LC)
                rs = rs_all[:, t:t + 1]
                nc.scalar.activation(ins_all[:, csl], sum4_all[:, csl], AF.Ln, scale=rs)
                nc.scalar.activation(G_all[:, csl], begE_all[:, csl], AF.Ln, scale=rs)
                lend = smpool.tile([P, LC], F32, tag="lend")
                nc.scalar.activation(lend[:], endE_all[:, csl], AF.Ln, scale=rs)
                nc.vector.tensor_scalar(E2_all[:, csl], lend[:], -EPS, None,
                                        AluOpType.min)

            # ---------------- E2 transpose + DRAM-broadcast -------------------
            E2_b = wpool.tile([P, LC * S], BF16, tag="e2_b")
            A_b = wpool.tile([P, LC * S], F32, tag="a_b")
            for t in range(NST):
                csl = slice(t * LC, (t + 1) * LC)
                tp2 = pss.tile([P, 512], F32, tag="ps_small")
                nc.tensor.transpose(tp2[:LC, :P], E2_all[:, csl], eye_sb[:])
                nc.scalar.activation(E2_colT[:, t * P:(t + 1) * P], tp2[:LC, :P],
                                     AF.Copy)
            dma_w_e2 = nc.sync.dma_start(out=e2_row_d[:], in_=E2_colT[:])
            dma_r_e2 = nc.sync.dma_start(
                out=E2_b[:], in_=e2_row_d[:].rearrange("l j -> (l j)").partition_broadcast(P))
            _add_dep_helper(dma_r_e2.ins, dma_w_e2.ins, True, "e2 row RAW via dram")

            # ---------------- cumsum over seq (exclusive), de-serialized ------
            # colsums for all tiles in one matmul -> (1, NST*LC)
            cs_ps = pss.tile([P, 512], F32, tag="ps_small")
            nc.tensor.matmul(cs_ps[:1, :NST * LC], ones_col[:], ins_all[:],
                             start=True, stop=True)
            cs_row = smpool.tile([1, NST * LC], F32, tag="cs_row")
            nc.scalar.activation(cs_row[:], cs_ps[:1, :NST * LC], AF.Copy)
            # inclusive prefix over t (log-shift adds), then use shifted reads
            pre = [cs_row]
            for lev, sh in enumerate((LC, 2 * LC, 4 * LC)):
                nxt = smpool.tile([1, NST * LC], F32, tag="pre%d" % lev)
                nc.vector.tensor_copy(nxt[:, :sh], pre[-1][:, :sh])
                nc.vector.tensor_tensor(nxt[:, sh:], pre[-1][:, sh:],
                                        pre[-1][:, :NST * LC - sh], AluOpType.add)
                pre.append(nxt)
            inc_pref = pre[-1]   # inclusive prefix of colsums over t

            for t in range(NST):
                csl = slice(t * LC, (t + 1) * LC)
                cum_ps = pss.tile([P, 512], F32, tag="ps_small")
                nc.tensor.matmul(cum_ps[:, :LC], ut_sb[:], ins_all[:, csl],
                                 start=True, stop=t != 0)
                if t > 0:
                    nc.tensor.matmul(cum_ps[:, :LC], ones_row[:],
                                     inc_pref[:, (t - 1) * LC: t * LC],
                                     start=False, stop=True)
                nc.scalar.activation(C_all[:, csl], cum_ps[:, :LC], AF.Copy)

            ncs_all = wpool.tile([P, NST * LC], F32, tag="ncs_all")
            nc.vector.tensor_scalar(ncs_all[:], C_all[:], -1.0, None, AluOpType.mult)
            A_incl = wpool.tile([P, NST * LC], F32, tag="a_incl")
            nc.vector.tensor_tensor(A_incl[:], C_all[:], ins_all[:], AluOpType.add)
            for t in range(NST):
                csl = slice(t * LC, (t + 1) * LC)
                tp = pss.tile([P, 512], F32, tag="ps_small")
                nc.tensor.transpose(tp[:LC, :P], A_incl[:, csl], eye_sb[:])
                nc.scalar.activation(A_colT[:, t * P:(t + 1) * P], tp[:LC, :P], AF.Copy)
            dma_w_a = nc.sync.dma_start(out=a_row_d[:], in_=A_colT[:])
            for g in range(4):
                lg = slice(g * 2 * S, (g + 1) * 2 * S)
                dma_r_a = nc.sync.dma_start(
                    out=A_b[:, lg],
                    in_=a_row_d[g * 2:(g + 1) * 2, :].rearrange("l j -> (l j)").partition_broadcast(P))
                _add_dep_helper(dma_r_a.ins, dma_w_a.ins, True, "a row RAW via dram")

            # ---------------- main span sweep (l-major, bf16) ----------------
            out3 = out[:].rearrange("(t p) f -> t p f", p=P)
            E2_b3 = E2_b[:].rearrange("p (l j) -> p l j", l=LC)
            for t in range(NST):
                i0 = t * P
                W = S - i0
                e2m = upool.tile([P, LC * P], OUT_DT, tag="e2m")
                nc.vector.tensor_tensor(e2m[:], mask_sb[:], E2_b3[:, :, i0:i0 + P],
                                        AluOpType.min)
                u = upool.tile([P, LC * W], OUT_DT, tag="u")
                for l in range(LC):
                    cs = C_all[:, t * LC + l: t * LC + l + 1]
                    gs = G_all[:, t * LC + l: t * LC + l + 1]
                    if l < ACT_SPLIT:
                        # ScalarE computes A - C (Identity with per-partition
                        # bias) into bf16; DVE then min's with G at 4x mode.
                        tsub = upool.tile([P, W], OUT_DT, tag="tsub")
                        nc.scalar.activation(tsub[:], A_b[:, l * S + i0:(l + 1) * S],
                                             AF.Identity, bias=ncs_all[:, t * LC + l: t * LC + l + 1])
                        nc.vector.tensor_scalar(u[:, l * W:(l + 1) * W], tsub[:],
                                                gs, None, AluOpType.min)
                    else:
                        nc.vector.tensor_scalar(
                            u[:, l * W:(l + 1) * W],
                            A_b[:, l * S + i0:(l + 1) * S],
                            cs, gs, AluOpType.subtract, AluOpType.min)
                oc = opool.tile([P, LC * W], OUT_DT, tag="oc")
                oc3 = oc[:].rearrange("p (l j) -> p l j", j=W)
                u3 = u[:].rearrange("p (l j) -> p l j", j=W)
                e2m3 = e2m[:].rearrange("p (l j) -> p l j", j=P)
                nc.vector.tensor_tensor(oc3[:, :, 0:P], u3[:, :, 0:P], e2m3,
                                        AluOpType.min)
                if W > P:
                    nc.vector.tensor_tensor(oc3[:, :, P:W], u3[:, :, P:W],
                                            E2_b3[:, :, i0 + P:S], AluOpType.min)
                dst = out3[t, :, :].rearrange("p (l j) -> p l j", l=LC)[:, :, i0:S]
                nc.sync.dma_start(out=dst, in_=oc3)

    nc.compile()
    return nc


def _host_inputs(x, W, b):
    """Build per-core input maps. Core c: batch c//2, label half c%2."""
    x = np.asarray(x, dtype=np.float32)
    W = np.asarray(W, dtype=np.float32)
    b = np.asarray(b, dtype=np.float32)

    Wb = np.concatenate([W, b[None, :]], axis=0)          # (401, 65)
    eye = np.eye(P, dtype=np.float32)
    ut = np.triu(np.ones((P, P), np.float32), k=1)        # ut[k,i]=1 iff i>k
    jj = np.arange(P)[None, :] >= np.arange(P)[:, None]
    m = np.where(jj, np.float32(1e30), np.float32(NEG)).astype(np.float32)
    m = _to_out_dt(np.tile(m, (1, LC)))

    in_maps = []
    for c in range(8):
        bb, h = c // 2, c % 2
        cols = []
        for l in range(LC):
            base = 1 + 4 * (h * LC + l)
            cols.extend(range(base, base + 4))
        xTb = np.concatenate([x[bb].T, np.ones((1, S), np.float32)], axis=0)
        in_maps.append({
            "xTb": np.ascontiguousarray(xTb),
            "Wcat": np.ascontiguousarray(np.concatenate([Wb, Wb[:, cols]], axis=1)),
            "eye": eye, "ut": ut, "mask8": m,
        })
    return in_maps


def _to_out_dt(a):
    if OUT_DT == F32:
        return a.astype(np.float32)
    u = a.astype(np.float32).view(np.uint32)
    r = ((u >> 16) & 1) + 0x7FFF
    return ((u + r) >> 16).astype(np.uint16)


def _from_out_dt(a):
    if OUT_DT == F32:
        return a
    return (a.astype(np.uint32) << 16).view(np.float32)


def kernel(x, mask, W, b, _collect=None):
    global _CACHED_NC
    if _CACHED_NC is None:
        _CACHED_NC = _build()
    nc = _CACHED_NC
    in_maps = _host_inputs(x, W, b)
    res = run_bass_kernel_spmd(nc, in_maps, list(range(8)))
    if _collect is not None:
        _collect.append(res)
    outf = np.empty((B, S, S, NL), dtype=np.float32)
    for c in range(8):
        bb, h = c // 2, c % 2
        o = res.results[c]["out"]
        if o.dtype != np.float32:
            o = _from_out_dt(o.view(OUT_NP) if o.dtype != OUT_NP else o)
        o = o.reshape(S, LC, S)                       # [i, l, j]
        outf[bb, :, :, h * LC:(h + 1) * LC] = o.transpose(0, 2, 1)
    # constant lower triangle filled on host (device writes only j >= i0 of
    # each row tile; below-diagonal within the tile is masked on device)
    for i in range(1, S):
        i0 = (i // P) * P
        if i0 > 0:
            outf[:, i, :i0, :] = NEG
    return outf


# revision 9
# speedup vs baseline: 2.4432x; 1.0814x over previous
"""Trainium2 Bass kernel for BERTSpanNER boundary scores.

out[b,i,j,l] = min(cum[j+1,l]-cum[i,l], -EPS, begin[i,l], end[j,l]) on the
upper triangle (j>=i), else -1e9, where cum/begin/end derive from
log_softmax(x @ W + b) per label's I,B,L,U tag group.

Sharding: 8 cores = 4 batches x 2 label-halves (8 labels each). All cores run
one identical SPMD graph; per-core work differs only through input data (the
batch slice of x, and a label-permuted copy of W's columns).

Device writes only the computed upper-triangle region in an l-major (S, LC, S)
bf16 layout; the constant -1e9 lower triangle is filled on the host, which
also transposes to [i, j, l] and upcasts to f32.
"""
import os
import sys

for _p in ("/opt/trn_rl_repo", "/root/.axon_site/_ro/trn_rl_repo"):
    if os.path.isdir(_p) and _p not in sys.path:
        sys.path.insert(0, _p)

import numpy as np
import concourse.bacc as bacc
import concourse.mybir as mybir
from concourse.bass import _add_dep_helper
from concourse.tile import TileContext
from concourse.bass_utils import run_bass_kernel_spmd
from concourse.alu_op_type import AluOpType

F32 = mybir.dt.float32
BF16 = mybir.dt.bfloat16
AF = mybir.ActivationFunctionType

B, S, H, NL = 4, 1024, 400, 16
NT = 1 + 4 * NL          # 65
EPS = 1e-8
NEG = -1e9
P = 128
NST = S // P             # 8 seq tiles
LC = NL // 2             # 8 labels per core
KT = [128, 128, 128, 17]  # k-tiling of H+1=401 (padded to 128-partition tiles)
ACT_SPLIT = 5            # labels 0..4 take the ScalarE subtract path

OUT_DT = BF16            # device output dtype (host upcasts)
OUT_NP = np.dtype("uint16")

_CACHED_NC = None


def _build():
    nc = bacc.Bacc()
    NW = NT + 4 * LC
    NKT = len(KT)
    xTb = nc.declare_dram_parameter("xTb", [P, NKT * S], F32, isOutput=False)
    Wcat = nc.declare_dram_parameter("Wcat", [P, NKT * NW], F32, isOutput=False)
    eye = nc.declare_dram_parameter("eye", [P, P], F32, isOutput=False)
    ut = nc.declare_dram_parameter("ut", [P, P], F32, isOutput=False)    # ut[k,i]=1 if k<i
    mask8 = nc.declare_dram_parameter("mask8", [P, LC * P], OUT_DT, isOutput=False)
    out = nc.declare_dram_parameter("out", [S, LC * S], OUT_DT, isOutput=True)

    a_row_d = nc.dram_tensor("a_row_d", [LC, S], F32)
    e2_row_d = nc.dram_tensor("e2_row_d", [LC, S], BF16)

    with TileContext(nc) as tc:
        with tc.tile_pool(name="const", bufs=1) as cpool, \
             tc.tile_pool(name="work", bufs=1) as wpool, \
             tc.tile_pool(name="sm", bufs=8) as smpool, \
             tc.tile_pool(name="u", bufs=2) as upool, \
             tc.tile_pool(name="oc", bufs=2) as opool, \
             tc.tile_pool(name="ps_small", bufs=4, space="PSUM") as pss:

            # ---------------- input loads (single packed DMAs) ---------------
            xk_all = cpool.tile([P, NKT * S], F32, tag="xk_all")
            nc.sync.dma_start(out=xk_all[:], in_=xTb[:])
            wc_all = cpool.tile([P, NKT * NW], F32, tag="wc_all")
            nc.scalar.dma_start(out=wc_all[:], in_=Wcat[:])
            eye_sb = cpool.tile([P, P], F32, tag="eye")
            nc.scalar.dma_start(out=eye_sb[:], in_=eye[:])
            ut_sb = cpool.tile([P, P], F32, tag="ut")
            nc.gpsimd.dma_start(out=ut_sb[:], in_=ut[:])
            mask_sb = cpool.tile([P, LC * P], OUT_DT, tag="mask8")
            nc.gpsimd.dma_start(out=mask_sb[:], in_=mask8[:])

            ones_row = cpool.tile([1, P], F32, tag="ones_row")
            nc.vector.memset(ones_row[:], 1.0)
            ones_col = cpool.tile([P, 1], F32, tag="ones_col")
            nc.vector.memset(ones_col[:], 1.0)

            # ---------------- prologue phase 1: matmul + exp + partial sums ---
            C_all = wpool.tile([P, NST * LC], F32, tag="c_all")
            G_all = wpool.tile([P, NST * LC], F32, tag="g_all")
            ins_all = wpool.tile([P, NST * LC], F32, tag="ins_all")
            E2_all = wpool.tile([P, NST * LC], F32, tag="e2_all")
            A_colT = wpool.tile([LC, S], F32, tag="a_colt")
            E2_colT = wpool.tile([LC, S], BF16, tag="e2_colt")
            sum4_all = wpool.tile([P, NST * LC], F32, tag="sum4_all")
            begE_all = wpool.tile([P, NST * LC], F32, tag="bege_all")
            endE_all = wpool.tile([P, NST * LC], F32, tag="ende_all")
            rs_all = wpool.tile([P, NST], F32, tag="rs_all")

            for t in range(NST):
                sl = slice(t * P, (t + 1) * P)
                csl = slice(t * LC, (t + 1) * LC)
                ps97 = pss.tile([P, 512], F32, tag="ps_small")
                for ki, kt in enumerate(KT):
                    st, sp = ki == 0, ki == len(KT) - 1
                    nc.tensor.matmul(ps97[:, :NW],
                                     xk_all[0:kt, ki * S + t * P: ki * S + (t + 1) * P],
                                     wc_all[0:kt, ki * NW:(ki + 1) * NW],
                                     start=st, stop=sp)

                rowmax = smpool.tile([P, 1], F32, tag="rowmax")
                nc.vector.tensor_reduce(rowmax[:], ps97[:, :NT], mybir.AxisListType.X,
                                        AluOpType.max)
                nrm = smpool.tile([P, 1], F32, tag="nrm")
                nc.vector.tensor_scalar(nrm[:], rowmax[:], -1.0, None, AluOpType.mult)

                e97 = smpool.tile([P, NW], F32, tag="e97")
                nc.scalar.activation(e97[:], ps97[:, :NW], AF.Exp, bias=nrm[:])
                e65 = e97[:, :NT]
                elab = e97[:, NT:NW]

                ssum = smpool.tile([P, 1], F32, tag="ssum")
                nc.vector.tensor_reduce(ssum[:], e65[:], mybir.AxisListType.X,
                                        AluOpType.add)
                nc.vector.reciprocal(rs_all[:, t:t + 1], ssum[:])

                el = elab.rearrange("p (l k) -> p l k", k=4)
                t01 = smpool.tile([P, LC], F32, tag="t01")
                nc.vector.tensor_tensor(t01[:], el[:, :, 0], el[:, :, 1], AluOpType.add)
                t23 = smpool.tile([P, LC], F32, tag="t23")
                nc.vector.tensor_tensor(t23[:], el[:, :, 2], el[:, :, 3], AluOpType.add)
                nc.vector.tensor_tensor(sum4_all[:, csl], t01[:], t23[:], AluOpType.add)
                nc.vector.tensor_tensor(begE_all[:, csl], el[:, :, 1], el[:, :, 3],
                                        AluOpType.add)
                nc.vector.tensor_tensor(endE_all[:, csl], el[:, :, 2], el[:, :, 3],
                                        AluOpType.add)

            # ---------------- prologue phase 2: all the Ln's ------------------
            for t in range(NST):
                csl = slice(t * LC, (t + 1) * LC)
                rs = rs_all[:, t:t + 1]
                nc.scalar.activation(ins_all[:, csl], sum4_all[:, csl], AF.Ln, scale=rs)
                nc.scalar.activation(G_all[:, csl], begE_all[:, csl], AF.Ln, scale=rs)
                lend = smpool.tile([P, LC], F32, tag="lend")
                nc.scalar.activation(lend[:], endE_all[:, csl], AF.Ln, scale=rs)
                nc.vector.tensor_scalar(E2_all[:, csl], lend[:], -EPS, None,
                                        AluOpType.min)

            # ---------------- E2 transpose + DRAM-broadcast -------------------
            E2_b = wpool.tile([P, LC * S], BF16, tag="e2_b")
            A_b = wpool.tile([P, LC * S], F32, tag="a_b")
            for t in range(NST):
                csl = slice(t * LC, (t + 1) * LC)
                tp2 = pss.tile([P, 512], F32, tag="ps_small")
                nc.tensor.transpose(tp2[:LC, :P], E2_all[:, csl], eye_sb[:])
                nc.scalar.activation(E2_colT[:, t * P:(t + 1) * P], tp2[:LC, :P],
                                     AF.Copy)
            dma_w_e2 = nc.sync.dma_start(out=e2_row_d[:], in_=E2_colT[:])
            dma_r_e2 = nc.sync.dma_start(
                out=E2_b[:], in_=e2_row_d[:].rearrange("l j -> (l j)").partition_broadcast(P))
            _add_dep_helper(dma_r_e2.ins, dma_w_e2.ins, True, "e2 row RAW via dram")

            # ---------------- cumsum over seq (exclusive), de-serialized ------
            # colsums for all tiles in one matmul -> (1, NST*LC)
            cs_ps = pss.tile([P, 512], F32, tag="ps_small")
            nc.tensor.matmul(cs_ps[:1, :NST * LC], ones_col[:], ins_all[:],
                             start=True, stop=True)
            cs_row = smpool.tile([1, NST * LC], F32, tag="cs_row")
            nc.scalar.activation(cs_row[:], cs_ps[:1, :NST * LC], AF.Copy)
            # inclusive prefix over t (log-shift adds), then use shifted reads
            pre = [cs_row]
            for lev, sh in enumerate((LC, 2 * LC, 4 * LC)):
                nxt = smpool.tile([1, NST * LC], F32, tag="pre%d" % lev)
                nc.vector.tensor_copy(nxt[:, :sh], pre[-1][:, :sh])
                nc.vector.tensor_tensor(nxt[:, sh:], pre[-1][:, sh:],
                                        pre[-1][:, :NST * LC - sh], AluOpType.add)
                pre.append(nxt)
            inc_pref = pre[-1]   # inclusive prefix of colsums over t

            for t in range(NST):
                csl = slice(t * LC, (t + 1) * LC)
                cum_ps = pss.tile([P, 512], F32, tag="ps_small")
                nc.tensor.matmul(cum_ps[:, :LC], ut_sb[:], ins_all[:, csl],
                                 start=True, stop=t != 0)
                if t > 0:
                    nc.tensor.matmul(cum_ps[:, :LC], ones_row[:],
                                     inc_pref[:, (t - 1) * LC: t * LC],
                                     start=False, stop=True)
                nc.scalar.activation(C_all[:, csl], cum_ps[:, :LC], AF.Copy)

            ncs_all = wpool.tile([P, NST * LC], F32, tag="ncs_all")
            nc.vector.tensor_scalar(ncs_all[:], C_all[:], -1.0, None, AluOpType.mult)
            A_incl = wpool.tile([P, NST * LC], F32, tag="a_incl")
            nc.vector.tensor_tensor(A_incl[:], C_all[:], ins_all[:], AluOpType.add)
            for t in range(NST):
                csl = slice(t * LC, (t + 1) * LC)
                tp = pss.tile([P, 512], F32, tag="ps_small")
                nc.tensor.transpose(tp[:LC, :P], A_incl[:, csl], eye_sb[:])
                nc.scalar.activation(A_colT[:, t * P:(t + 1) * P], tp[:LC, :P], AF.Copy)
            dma_w_a = nc.sync.dma_start(out=a_row_d[:], in_=A_colT[:])
            for g in range(4):
                lg = slice(g * 2 * S, (g + 1) * 2 * S)
                dma_r_a = nc.sync.dma_start(
                    out=A_b[:, lg],
                    in_=a_row_d[g * 2:(g + 1) * 2, :].rearrange("l j -> (l j)").partition_broadcast(P))
                _add_dep_helper(dma_r_a.ins, dma_w_a.ins, True, "a row RAW via dram")

            # ---------------- main span sweep (l-major, bf16) ----------------
            out3 = out[:].rearrange("(t p) f -> t p f", p=P)
            E2_b3 = E2_b[:].rearrange("p (l j) -> p l j", l=LC)
            for t in range(NST):
                i0 = t * P
                W = S - i0
                e2m = upool.tile([P, LC * P], OUT_DT, tag="e2m")
                nc.vector.tensor_tensor(e2m[:], mask_sb[:], E2_b3[:, :, i0:i0 + P],
                                        AluOpType.min)
                u = upool.tile([P, LC * W], OUT_DT, tag="u")
                for l in range(LC):
                    cs = C_all[:, t * LC + l: t * LC + l + 1]
                    gs = G_all[:, t * LC + l: t * LC + l + 1]
                    if l < ACT_SPLIT:
                        # ScalarE computes A - C (Identity with per-partition
                        # bias) into bf16; DVE then min's with G at 4x mode.
                        tsub = upool.tile([P, W], OUT_DT, tag="tsub")
                        nc.scalar.activation(tsub[:], A_b[:, l * S + i0:(l + 1) * S],
                                             AF.Identity, bias=ncs_all[:, t * LC + l: t * LC + l + 1])
                        nc.vector.tensor_scalar(u[:, l * W:(l + 1) * W], tsub[:],
                                                gs, None, AluOpType.min)
                    else:
                        nc.vector.tensor_scalar(
                            u[:, l * W:(l + 1) * W],
                            A_b[:, l * S + i0:(l + 1) * S],
                            cs, gs, AluOpType.subtract, AluOpType.min)
                oc = opool.tile([P, LC * W], OUT_DT, tag="oc")
                oc3 = oc[:].rearrange("p (l j) -> p l j", j=W)
                u3 = u[:].rearrange("p (l j) -> p l j", j=W)
                e2m3 = e2m[:].rearrange("p (l j) -> p l j", j=P)
                nc.vector.tensor_tensor(oc3[:, :, 0:P], u3[:, :, 0:P], e2m3,
                                        AluOpType.min)
                if W > P:
                    nc.vector.tensor_tensor(oc3[:, :, P:W], u3[:, :, P:W],
                                            E2_b3[:, :, i0 + P:S], AluOpType.min)
                dst = out3[t, :, :].rearrange("p (l j) -> p l j", l=LC)[:, :, i0:S]
                nc.sync.dma_start(out=dst, in_=oc3)

    nc.compile()
    return nc


def _host_inputs(x, W, b):
    """Build per-core input maps. Core c: batch c//2, label half c%2."""
    x = np.asarray(x, dtype=np.float32)
    W = np.asarray(W, dtype=np.float32)
    b = np.asarray(b, dtype=np.float32)

    Wb = np.concatenate([W, b[None, :]], axis=0)          # (401, 65)
    eye = np.eye(P, dtype=np.float32)
    ut = np.triu(np.ones((P, P), np.float32), k=1)        # ut[k,i]=1 iff i>k
    jj = np.arange(P)[None, :] >= np.arange(P)[:, None]
    m = np.where(jj, np.float32(1e30), np.float32(NEG)).astype(np.float32)
    m = _to_out_dt(np.tile(m, (1, LC)))

    in_maps = []
    for c in range(8):
        bb, h = c // 2, c % 2
        cols = []
        for l in range(LC):
            base = 1 + 4 * (h * LC + l)
            cols.extend(range(base, base + 4))
        xTb = np.concatenate([x[bb].T, np.ones((1, S), np.float32)], axis=0)
        wcat = np.concatenate([Wb, Wb[:, cols]], axis=1)          # (401, 97)
        xp = np.zeros((4 * P, S), np.float32)
        xp[:H + 1] = xTb
        xp = np.ascontiguousarray(xp.reshape(4, P, S).transpose(1, 0, 2).reshape(P, 4 * S))
        wp = np.zeros((4 * P, wcat.shape[1]), np.float32)
        wp[:H + 1] = wcat
        wp = np.ascontiguousarray(wp.reshape(4, P, -1).transpose(1, 0, 2).reshape(P, -1))
        in_maps.append({
            "xTb": xp, "Wcat": wp,
            "eye": eye, "ut": ut, "mask8": m,
        })
    return in_maps


def _to_out_dt(a):
    if OUT_DT == F32:
        return a.astype(np.float32)
    u = a.astype(np.float32).view(np.uint32)
    r = ((u >> 16) & 1) + 0x7FFF
    return ((u + r) >> 16).astype(np.uint16)


def _from_out_dt(a):
    if OUT_DT == F32:
        return a
    return (a.astype(np.uint32) << 16).view(np.float32)


def kernel(x, mask, W, b, _collect=None):
    global _CACHED_NC
    if _CACHED_NC is None:
        _CACHED_NC = _build()
    nc = _CACHED_NC
    in_maps = _host_inputs(x, W, b)
    res = run_bass_kernel_spmd(nc, in_maps, list(range(8)))
    if _collect is not None:
        _collect.append(res)
    outf = np.empty((B, S, S, NL), dtype=np.float32)
    for c in range(8):
        bb, h = c // 2, c % 2
        o = res.results[c]["out"]
        if o.dtype != np.float32:
            o = _from_out_dt(o.view(OUT_NP) if o.dtype != OUT_NP else o)
        o = o.reshape(S, LC, S)                       # [i, l, j]
        outf[bb, :, :, h * LC:(h + 1) * LC] = o.transpose(0, 2, 1)
    # constant lower triangle filled on host (device writes only j >= i0 of
    # each row tile; below-diagonal within the tile is masked on device)
    for i in range(1, S):
        i0 = (i // P) * P
        if i0 > 0:
            outf[:, i, :i0, :] = NEG
    return outf
